# revision 5
# baseline (speedup 1.0000x reference)
"""GroupQueryAttention (softmax over the GROUP axis) on 8 trn2 NeuronCores.

Reference computation (B=2, S=2048, D=1024, G=8, h=128):
    q = hidden @ Wq + bq ; k = hidden @ Wk + bk ; v = hidden @ Wv + bv
    scores[b,n,m,g] = sum_h q[b,n,g,h] k[b,m,g,h] / sqrt(D)
    probs = softmax(scores, axis=g)            # couples groups per (n,m)
    ctx[b,n,g,h] = sum_m probs[b,n,m,g] v[b,m,g,h]

Sharding: 2 batches x 4 query-blocks of 512 = 8 cores. The softmax over
g is local per core. Each core recomputes its batch's full K,V (8.6
GFLOP/core extra) to avoid cross-core collectives; all matmuls run in
bf16 with fp32 PSUM accumulation.

Per-core layouts (SBUF partition dim first):
    XT   (1024, 2048) bf16  hidden[b].T, columns rolled so that this
                            core's 512 query columns come first
    Q^T  (128, 8, 512)      per h-tile g: rows h, cols n (scaled 1/32, +bq)
    K^T  (128, 8, 2048)     per h-tile g: rows h, cols m (+bk)
    V    (128, 16, 1024)    per m-tile: rows m, cols d=g*128+h (+bv via
                            rank-1 ones matmul)
    scores^T, probs^T kept (m, n); ctx accumulated transposed (h, n).
Output: ctxT (1024, 512) fp32 per core; host transposes and concats.
"""

import os

os.environ.setdefault("JAX_COMPILATION_CACHE_DIR", "/tmp/jax_comp_cache")

import numpy as np
import ml_dtypes

import concourse.bass as bass
import concourse.mybir as mybir
import concourse.tile as tile
from concourse import bacc
from concourse.bass_utils import run_bass_kernel_spmd

BF16 = mybir.dt.bfloat16
F32 = mybir.dt.float32

B, S, D, G = 2, 2048, 1024, 8
H = D // G          # 128, group head dim
NQ = S // 4         # 512 queries per core
MT = S // 128       # 16 key m-tiles
CN = 256            # n-chunk (queries per attention pass)
NP = NQ // CN       # 2 passes
SCALE = 1.0 / np.sqrt(np.float32(D))  # 1/32

_CACHE = {}


def _build():
    nc = bacc.Bacc()

    xt_d = nc.dram_tensor("xt", [D, S], BF16, kind="ExternalInput")
    wq_d = nc.dram_tensor("wq", [D, D], BF16, kind="ExternalInput")
    wk_d = nc.dram_tensor("wk", [D, D], BF16, kind="ExternalInput")
    wv_d = nc.dram_tensor("wv", [D, D], BF16, kind="ExternalInput")
    bqs_d = nc.dram_tensor("bqs", [128, G], F32, kind="ExternalInput")
    bks_d = nc.dram_tensor("bks", [128, G], F32, kind="ExternalInput")
    bvt_d = nc.dram_tensor("bvt", [1, D], BF16, kind="ExternalInput")
    out_d = nc.dram_tensor("ctxT", [D, NQ], F32, kind="ExternalOutput")

    with tile.TileContext(nc) as tc:
        with (
            tc.tile_pool(name="big", bufs=1) as big,
            tc.tile_pool(name="small", bufs=1) as small,
            tc.tile_pool(name="ework", bufs=2) as ework,
            tc.tile_pool(name="zwork", bufs=2) as zwork,
            tc.tile_pool(name="pj", bufs=2, space="PSUM") as pj,
            tc.tile_pool(name="sc", bufs=1, space="PSUM") as scp,
            tc.tile_pool(name="cx", bufs=1, space="PSUM") as cxp,
        ):
            # ---- load inputs, ordered/chunked by first consumer -------------
            xt_r = xt_d.rearrange("(t p) m -> p t m", p=128)
            xt_s = big.tile([128, 8, S], BF16)       # [p, dt, m] d = dt*128+p
            wq_s = big.tile([128, 8, D], BF16)
            nc.sync.dma_start(wq_s[:], wq_d.rearrange("(t p) o -> p t o", p=128))
            nc.sync.dma_start(xt_s[:, :, 0:512], xt_r[:, :, 0:512])
            bqs_s = small.tile([128, G], F32)
            nc.gpsimd.dma_start(bqs_s[:], bqs_d[:])
            wk_s = big.tile([128, 8, D], BF16)
            nc.sync.dma_start(wk_s[:], wk_d.rearrange("(t p) o -> p t o", p=128))
            bks_s = small.tile([128, G], F32)
            nc.gpsimd.dma_start(bks_s[:], bks_d[:])
            wv_s = big.tile([128, 8, D], BF16)
            nc.sync.dma_start(wv_s[:], wv_d.rearrange("(t p) o -> p t o", p=128))
            bvt_s = small.tile([1, D], BF16)
            nc.gpsimd.dma_start(bvt_s[:], bvt_d[:])
            nc.sync.dma_start(xt_s[:, :, 512:2048], xt_r[:, :, 512:2048])
            ones_s = small.tile([1, 128], BF16)
            nc.vector.memset(ones_s[:], 1.0)

            kt_s = big.tile([128, G, S], BF16)       # [h, g, m]
            v_s = big.tile([128, MT, D], BF16)       # [m, mt, g*128+h]
            qt_s = big.tile([128, G, NQ], BF16)      # [h, g, n]
            ctxt_s = big.tile([128, G, NQ], F32)     # [h, g, n]

            ident = mybir.ActivationFunctionType.Identity
            expf = mybir.ActivationFunctionType.Exp

            # ---- Q^T projection (queries are XT columns 0:NQ) ---------------
            for g in range(G):
                qp = pj.tile([128, NQ], F32, tag="pj")
                for dt in range(8):
                    nc.tensor.matmul(
                        qp[:],
                        wq_s[:, dt, g * 128 : (g + 1) * 128],
                        xt_s[:, dt, 0:NQ],
                        start=(dt == 0),
                        stop=(dt == 7),
                    )
                nc.scalar.activation(
                    qt_s[:, g, :], qp[:], ident,
                    bias=bqs_s[:, g : g + 1], scale=float(SCALE),
                )

            # ---- main: produce K/V by m-chunk, run attention pass 0 ---------
            def kv_chunk(mc):
                # K^T columns mc*512 .. +512 for all 8 groups
                for g in range(G):
                    kp = pj.tile([128, 512], F32, tag="pj")
                    for dt in range(8):
                        nc.tensor.matmul(
                            kp[:],
                            wk_s[:, dt, g * 128 : (g + 1) * 128],
                            xt_s[:, dt, mc * 512 : (mc + 1) * 512],
                            start=(dt == 0),
                            stop=(dt == 7),
                        )
                    nc.scalar.activation(
                        kt_s[:, g, mc * 512 : (mc + 1) * 512], kp[:], ident,
                        bias=bks_s[:, g : g + 1],
                    )
                # V rows (4 m-tiles of 128) x full D
                for mt in range(4 * mc, 4 * mc + 4):
                    for hc in range(2):
                        vp = pj.tile([128, 512], F32, tag="pj")
                        for dt in range(8):
                            nc.tensor.matmul(
                                vp[:],
                                xt_s[:, dt, mt * 128 : (mt + 1) * 128],
                                wv_s[:, dt, hc * 512 : (hc + 1) * 512],
                                start=(dt == 0),
                                stop=False,
                            )
                        nc.tensor.matmul(
                            vp[:],
                            ones_s[:],
                            bvt_s[:, hc * 512 : (hc + 1) * 512],
                            start=False,
                            stop=True,
                        )
                        nc.scalar.activation(
                            v_s[:, mt, hc * 512 : (hc + 1) * 512], vp[:], ident
                        )

            def scores_softmax(np_, mt, fill=None):
                """scores^T + exp + group-softmax normalization for one
                m-tile; returns the normalized P tile (in place of E).
                fill(half) emits deferred PE work after each score half."""
                n0 = np_ * CN
                e_s = ework.tile([128, G, CN], BF16, tag="e")
                for half in range(2):
                    sp = scp.tile([128, 4, CN], F32, tag="sc")
                    for gl in range(4):
                        g = half * 4 + gl
                        nc.tensor.matmul(
                            sp[:, gl, :],
                            kt_s[:, g, mt * 128 : (mt + 1) * 128],
                            qt_s[:, g, n0 : n0 + CN],
                            start=True,
                            stop=True,
                        )
                    nc.scalar.activation(
                        e_s[:, half * 4 : half * 4 + 4, :], sp[:], expf
                    )
                    if fill is not None:
                        fill(half)
                # Z = sum over g (pairwise tree), P = E / Z
                t1 = zwork.tile([128, 4, CN], BF16, tag="t1")
                nc.vector.tensor_add(t1[:], e_s[:, 0:4, :], e_s[:, 4:8, :])
                t2 = zwork.tile([128, 2, CN], BF16, tag="t2")
                nc.vector.tensor_add(t2[:], t1[:, 0:2, :], t1[:, 2:4, :])
                z32 = zwork.tile([128, CN], F32, tag="z32")
                nc.vector.tensor_add(z32[:], t2[:, 0, :], t2[:, 1, :])
                w32 = zwork.tile([128, CN], F32, tag="w32")
                nc.vector.reciprocal_approx_fast(out=w32[:], in_=z32[:])
                wb = zwork.tile([128, CN], BF16, tag="wb")
                nc.vector.tensor_copy(wb[:], w32[:])
                wb_b = bass.AP(
                    tensor=wb.tensor, offset=wb.offset,
                    ap=[wb.ap[0], [0, G], wb.ap[1]],
                )
                wb_b4 = bass.AP(
                    tensor=wb.tensor, offset=wb.offset,
                    ap=[wb.ap[0], [0, 4], wb.ap[1]],
                )
                nc.vector.tensor_mul(e_s[:, 0:4, :], e_s[:, 0:4, :], wb_b4)
                nc.gpsimd.tensor_mul(e_s[:, 4:8, :], e_s[:, 4:8, :], wb_b4)
                return e_s

            def ctx_mms(mt, e_s, ctx_acc, gs=range(G)):
                # ctx^T accumulation: out[h, n] += V_g^T @ P_g^T
                # Two groups share each 2KB PSUM bank. start=True resets the
                # whole bank's has_written bits, so only the first group of
                # each bank pair may issue it; the second group's first write
                # lands on cleared bits and overwrites, later writes accumulate.
                for g in gs:
                    nc.tensor.matmul(
                        ctx_acc[:, g, :],
                        v_s[:, mt, g * 128 : (g + 1) * 128],
                        e_s[:, g, :],
                        start=(mt == 0 and g % 2 == 0),
                        stop=(mt == MT - 1),
                        skip_group_check=True,
                    )

            def emit_pass(np_, ctx_acc, with_kv=False):
                # software pipeline: ctx matmuls for m-tile mt are emitted
                # after scores(mt+1), so the PE streams the next m-tile's
                # scores instead of stalling on the DVE softmax chain.
                pend = None
                for mt in range(MT):
                    if with_kv and mt % 4 == 0:
                        kv_chunk(mt // 4)

                    def fill(half, _p=pend):
                        if _p is not None:
                            ctx_mms(_p[0], _p[1], ctx_acc,
                                    gs=range(half * 4, half * 4 + 4))

                    e_s = scores_softmax(np_, mt, fill)
                    pend = (mt, e_s)
                ctx_mms(pend[0], pend[1], ctx_acc)
                # evacuate in halves so the next pass's first banks free early
                n0 = np_ * CN
                nc.any.tensor_copy(
                    ctxt_s[:, 0:4, n0 : n0 + CN], ctx_acc[:, 0:4, :]
                )
                nc.any.tensor_copy(
                    ctxt_s[:, 4:8, n0 : n0 + CN], ctx_acc[:, 4:8, :]
                )
                nc.sync.dma_start(
                    out_d.rearrange("(t p) n -> p t n", p=128)[:, :, n0 : n0 + CN],
                    ctxt_s[:, :, n0 : n0 + CN],
                )

            for np_ in range(NP):
                ctx_acc = cxp.tile([128, G, CN], F32, tag="cx")
                emit_pass(np_, ctx_acc, with_kv=(np_ == 0))

    nc.compile()
    return nc


def _prep_inputs(hidden_states, Wq, bq, Wk, bk, Wv, bv):
    bf = ml_dtypes.bfloat16
    wq_b = np.ascontiguousarray(Wq, dtype=np.float32).astype(bf)
    wk_b = np.ascontiguousarray(Wk, dtype=np.float32).astype(bf)
    wv_b = np.ascontiguousarray(Wv, dtype=np.float32).astype(bf)
    bqs = np.ascontiguousarray(
        (np.asarray(bq, np.float32) * SCALE).reshape(G, 128).T
    )
    bks = np.ascontiguousarray(np.asarray(bk, np.float32).reshape(G, 128).T)
    bvt = np.asarray(bv, np.float32).astype(bf).reshape(1, D)

    in_maps = []
    for core in range(8):
        b, j = divmod(core, 4)
        xt = np.asarray(hidden_states[b], np.float32).T  # (D, S)
        xt = np.roll(xt, -j * NQ, axis=1)                # queries first
        in_maps.append(
            {
                "xt": np.ascontiguousarray(xt).astype(bf),
                "wq": wq_b, "wk": wk_b, "wv": wv_b,
                "bqs": bqs, "bks": bks, "bvt": bvt,
            }
        )
    return in_maps


def kernel(hidden_states, Wq, bq, Wk, bk, Wv, bv, _trace=False, _tmpdir=None):
    if "nc" not in _CACHE:
        _CACHE["nc"] = _build()
    nc = _CACHE["nc"]
    in_maps = _prep_inputs(hidden_states, Wq, bq, Wk, bk, Wv, bv)
    res = run_bass_kernel_spmd(
        nc, in_maps, list(range(8)), trace=_trace,
        **({"tmpdir": _tmpdir} if _tmpdir else {}),
    )
    _CACHE["last_result"] = res
    out = np.empty((B, S, D), np.float32)
    for core in range(8):
        b, j = divmod(core, 4)
        out[b, j * NQ : (j + 1) * NQ, :] = res.results[core]["ctxT"].T
    return out


# revision 6
# speedup vs baseline: 1.1701x; 1.1701x over previous
"""GroupQueryAttention (softmax over the GROUP axis) on 8 trn2 NeuronCores.

Reference computation (B=2, S=2048, D=1024, G=8, h=128):
    q = hidden @ Wq + bq ; k = hidden @ Wk + bk ; v = hidden @ Wv + bv
    scores[b,n,m,g] = sum_h q[b,n,g,h] k[b,m,g,h] / sqrt(D)
    probs = softmax(scores, axis=g)            # couples groups per (n,m)
    ctx[b,n,g,h] = sum_m probs[b,n,m,g] v[b,m,g,h]

Sharding: 2 batches x 4 query-blocks of 512 = 8 cores. The softmax over
g is local per core. Each core recomputes its batch's full K,V (8.6
GFLOP/core extra) to avoid cross-core collectives; all matmuls run in
bf16 with fp32 PSUM accumulation.

Per-core layouts (SBUF partition dim first):
    XT   (1024, 2048) bf16  hidden[b].T, columns rolled so that this
                            core's 512 query columns come first
    Q^T  (128, 8, 512)      per h-tile g: rows h, cols n (scaled 1/32, +bq)
    K^T  (128, 8, 2048)     per h-tile g: rows h, cols m (+bk)
    V    (128, 16, 1024)    per m-tile: rows m, cols d=g*128+h (+bv via
                            rank-1 ones matmul)
    scores^T, probs^T kept (m, n); ctx accumulated transposed (h, n).
Output: ctxT (1024, 512) fp32 per core; host transposes and concats.
"""

import os

os.environ.setdefault("JAX_COMPILATION_CACHE_DIR", "/tmp/jax_comp_cache")

import numpy as np
import ml_dtypes

import concourse.bass as bass
import concourse.mybir as mybir
import concourse.tile as tile
from concourse import bacc
from concourse.bass_utils import run_bass_kernel_spmd

BF16 = mybir.dt.bfloat16
F32 = mybir.dt.float32

B, S, D, G = 2, 2048, 1024, 8
H = D // G          # 128, group head dim
NQ = S // 4         # 512 queries per core
MT = S // 128       # 16 key m-tiles
CN = 256            # n-chunk (queries per attention pass)
NP = NQ // CN       # 2 passes
SCALE = 1.0 / np.sqrt(np.float32(D))  # 1/32

_CACHE = {}


def _build():
    nc = bacc.Bacc()

    xt_d = nc.dram_tensor("xt", [D, S], BF16, kind="ExternalInput")
    wq_d = nc.dram_tensor("wq", [D, D], BF16, kind="ExternalInput")
    wk_d = nc.dram_tensor("wk", [D, D], BF16, kind="ExternalInput")
    wv_d = nc.dram_tensor("wv", [D, D], BF16, kind="ExternalInput")
    bqs_d = nc.dram_tensor("bqs", [128, G], F32, kind="ExternalInput")
    bks_d = nc.dram_tensor("bks", [128, G], F32, kind="ExternalInput")
    bvt_d = nc.dram_tensor("bvt", [1, D], BF16, kind="ExternalInput")
    out_d = nc.dram_tensor("ctxT", [D, NQ], F32, kind="ExternalOutput")

    with tile.TileContext(nc) as tc:
        with (
            tc.tile_pool(name="big", bufs=1) as big,
            tc.tile_pool(name="small", bufs=1) as small,
            tc.tile_pool(name="ework", bufs=3) as ework,
            tc.tile_pool(name="zwork", bufs=3) as zwork,
            tc.tile_pool(name="pj", bufs=2, space="PSUM") as pj,
            tc.tile_pool(name="sc", bufs=1, space="PSUM") as scp,
            tc.tile_pool(name="cx", bufs=1, space="PSUM") as cxp,
        ):
            # ---- load inputs, ordered/chunked by first consumer -------------
            xt_r = xt_d.rearrange("(t p) m -> p t m", p=128)
            xt_s = big.tile([128, 8, S], BF16)       # [p, dt, m] d = dt*128+p
            wq_s = big.tile([128, 8, D], BF16)
            wq_r = wq_d.rearrange("(t p) o -> p t o", p=128)
            nc.sync.dma_start(wq_s[:, :, 0:512], wq_r[:, :, 0:512])
            nc.sync.dma_start(xt_s[:, :, 0:512], xt_r[:, :, 0:512])
            nc.sync.dma_start(wq_s[:, :, 512:1024], wq_r[:, :, 512:1024])
            bqs_s = small.tile([128, G], F32)
            nc.gpsimd.dma_start(bqs_s[:], bqs_d[:])
            wk_s = big.tile([128, 8, D], BF16)
            nc.sync.dma_start(wk_s[:], wk_d.rearrange("(t p) o -> p t o", p=128))
            bks_s = small.tile([128, G], F32)
            nc.gpsimd.dma_start(bks_s[:], bks_d[:])
            wv_s = big.tile([128, 8, D], BF16)
            nc.sync.dma_start(wv_s[:], wv_d.rearrange("(t p) o -> p t o", p=128))
            bvt_s = small.tile([1, D], BF16)
            nc.gpsimd.dma_start(bvt_s[:], bvt_d[:])
            nc.sync.dma_start(xt_s[:, :, 512:2048], xt_r[:, :, 512:2048])
            ones_s = small.tile([1, 128], BF16)
            nc.vector.memset(ones_s[:], 1.0)

            kt_s = big.tile([128, G, S], BF16)       # [h, g, m]
            v_s = big.tile([128, MT, D], BF16)       # [m, mt, g*128+h]
            qt_s = big.tile([128, G, NQ], BF16)      # [h, g, n]
            ctxt_s = big.tile([128, G, NQ], F32)     # [h, g, n]

            ident = mybir.ActivationFunctionType.Identity
            expf = mybir.ActivationFunctionType.Exp

            # ---- Q^T projection (queries are XT columns 0:NQ) ---------------
            for g in range(G):
                qp = pj.tile([128, NQ], F32, tag="pj")
                for dt in range(8):
                    nc.tensor.matmul(
                        qp[:],
                        wq_s[:, dt, g * 128 : (g + 1) * 128],
                        xt_s[:, dt, 0:NQ],
                        start=(dt == 0),
                        stop=(dt == 7),
                    )
                nc.scalar.activation(
                    qt_s[:, g, :], qp[:], ident,
                    bias=bqs_s[:, g : g + 1], scale=float(SCALE),
                )

            # ---- main: produce K/V by m-chunk, run attention pass 0 ---------
            def kv_chunk(mc):
                # K^T columns mc*512 .. +512 for all 8 groups
                for g in range(G):
                    kp = pj.tile([128, 512], F32, tag="pj")
                    for dt in range(8):
                        nc.tensor.matmul(
                            kp[:],
                            wk_s[:, dt, g * 128 : (g + 1) * 128],
                            xt_s[:, dt, mc * 512 : (mc + 1) * 512],
                            start=(dt == 0),
                            stop=(dt == 7),
                        )
                    nc.scalar.activation(
                        kt_s[:, g, mc * 512 : (mc + 1) * 512], kp[:], ident,
                        bias=bks_s[:, g : g + 1],
                    )
                # V rows (4 m-tiles of 128) x full D
                for mt in range(4 * mc, 4 * mc + 4):
                    for hc in range(2):
                        vp = pj.tile([128, 512], F32, tag="pj")
                        for dt in range(8):
                            nc.tensor.matmul(
                                vp[:],
                                xt_s[:, dt, mt * 128 : (mt + 1) * 128],
                                wv_s[:, dt, hc * 512 : (hc + 1) * 512],
                                start=(dt == 0),
                                stop=False,
                            )
                        nc.tensor.matmul(
                            vp[:],
                            ones_s[:],
                            bvt_s[:, hc * 512 : (hc + 1) * 512],
                            start=False,
                            stop=True,
                        )
                        nc.scalar.activation(
                            v_s[:, mt, hc * 512 : (hc + 1) * 512], vp[:], ident
                        )

            def scores_softmax(np_, mt, fill=None):
                """scores^T + exp + group-softmax normalization for one
                m-tile; returns the normalized P tile (in place of E).
                fill(half) emits deferred PE work after each score half."""
                n0 = np_ * CN
                e_s = ework.tile([128, G, CN], BF16, tag="e")
                for half in range(2):
                    sp = scp.tile([128, 4, CN], F32, tag="sc")
                    for gl in range(4):
                        g = half * 4 + gl
                        nc.tensor.matmul(
                            sp[:, gl, :],
                            kt_s[:, g, mt * 128 : (mt + 1) * 128],
                            qt_s[:, g, n0 : n0 + CN],
                            start=True,
                            stop=True,
                        )
                    nc.scalar.activation(
                        e_s[:, half * 4 : half * 4 + 4, :], sp[:], expf
                    )
                    if fill is not None:
                        fill(half)
                # Z = sum over g (pairwise tree), P = E / Z
                t1 = zwork.tile([128, 4, CN], BF16, tag="t1")
                nc.vector.tensor_add(t1[:], e_s[:, 0:4, :], e_s[:, 4:8, :])
                t2 = zwork.tile([128, 2, CN], BF16, tag="t2")
                nc.vector.tensor_add(t2[:], t1[:, 0:2, :], t1[:, 2:4, :])
                z32 = zwork.tile([128, CN], F32, tag="z32")
                nc.vector.tensor_add(z32[:], t2[:, 0, :], t2[:, 1, :])
                w32 = zwork.tile([128, CN], F32, tag="w32")
                nc.vector.reciprocal_approx_fast(out=w32[:], in_=z32[:])
                wb = zwork.tile([128, CN], BF16, tag="wb")
                nc.vector.tensor_copy(wb[:], w32[:])
                wb_b = bass.AP(
                    tensor=wb.tensor, offset=wb.offset,
                    ap=[wb.ap[0], [0, G], wb.ap[1]],
                )
                nc.vector.tensor_mul(e_s[:], e_s[:], wb_b)
                return e_s

            def ctx_mms(mt, e_s, ctx_acc, gs=range(G)):
                # ctx^T accumulation: out[h, n] += V_g^T @ P_g^T
                # Two groups share each 2KB PSUM bank. start=True resets the
                # whole bank's has_written bits, so only the first group of
                # each bank pair may issue it; the second group's first write
                # lands on cleared bits and overwrites, later writes accumulate.
                for g in gs:
                    nc.tensor.matmul(
                        ctx_acc[:, g, :],
                        v_s[:, mt, g * 128 : (g + 1) * 128],
                        e_s[:, g, :],
                        start=(mt == 0 and g % 2 == 0),
                        stop=(mt == MT - 1),
                        skip_group_check=True,
                    )

            def emit_pass(np_, ctx_acc, with_kv=False):
                # software pipeline: ctx matmuls for m-tile mt are emitted
                # after scores(mt+1), so the PE streams the next m-tile's
                # scores instead of stalling on the DVE softmax chain.
                pend = None
                for mt in range(MT):
                    if with_kv and mt % 4 == 0:
                        kv_chunk(mt // 4)

                    def fill(half, _p=pend):
                        if _p is not None:
                            ctx_mms(_p[0], _p[1], ctx_acc,
                                    gs=range(half * 4, half * 4 + 4))

                    e_s = scores_softmax(np_, mt, fill)
                    pend = (mt, e_s)
                ctx_mms(pend[0], pend[1], ctx_acc)
                # evacuate in halves so the next pass's first banks free early
                n0 = np_ * CN
                out_r = out_d.rearrange("(t p) n -> p t n", p=128)
                nc.any.tensor_copy(
                    ctxt_s[:, 0:4, n0 : n0 + CN], ctx_acc[:, 0:4, :]
                )
                nc.sync.dma_start(
                    out_r[:, 0:4, n0 : n0 + CN], ctxt_s[:, 0:4, n0 : n0 + CN]
                )
                nc.any.tensor_copy(
                    ctxt_s[:, 4:8, n0 : n0 + CN], ctx_acc[:, 4:8, :]
                )
                nc.sync.dma_start(
                    out_r[:, 4:8, n0 : n0 + CN], ctxt_s[:, 4:8, n0 : n0 + CN]
                )

            for np_ in range(NP):
                ctx_acc = cxp.tile([128, G, CN], F32, tag="cx")
                emit_pass(np_, ctx_acc, with_kv=(np_ == 0))

    nc.compile()
    return nc


def _prep_inputs(hidden_states, Wq, bq, Wk, bk, Wv, bv):
    bf = ml_dtypes.bfloat16
    wq_b = np.ascontiguousarray(Wq, dtype=np.float32).astype(bf)
    wk_b = np.ascontiguousarray(Wk, dtype=np.float32).astype(bf)
    wv_b = np.ascontiguousarray(Wv, dtype=np.float32).astype(bf)
    bqs = np.ascontiguousarray(
        (np.asarray(bq, np.float32) * SCALE).reshape(G, 128).T
    )
    bks = np.ascontiguousarray(np.asarray(bk, np.float32).reshape(G, 128).T)
    bvt = np.asarray(bv, np.float32).astype(bf).reshape(1, D)

    in_maps = []
    for core in range(8):
        b, j = divmod(core, 4)
        xt = np.asarray(hidden_states[b], np.float32).T  # (D, S)
        xt = np.roll(xt, -j * NQ, axis=1)                # queries first
        in_maps.append(
            {
                "xt": np.ascontiguousarray(xt).astype(bf),
                "wq": wq_b, "wk": wk_b, "wv": wv_b,
                "bqs": bqs, "bks": bks, "bvt": bvt,
            }
        )
    return in_maps


def kernel(hidden_states, Wq, bq, Wk, bk, Wv, bv, _trace=False, _tmpdir=None):
    if "nc" not in _CACHE:
        _CACHE["nc"] = _build()
    nc = _CACHE["nc"]
    in_maps = _prep_inputs(hidden_states, Wq, bq, Wk, bk, Wv, bv)
    res = run_bass_kernel_spmd(
        nc, in_maps, list(range(8)), trace=_trace,
        **({"tmpdir": _tmpdir} if _tmpdir else {}),
    )
    _CACHE["last_result"] = res
    out = np.empty((B, S, D), np.float32)
    for core in range(8):
        b, j = divmod(core, 4)
        out[b, j * NQ : (j + 1) * NQ, :] = res.results[core]["ctxT"].T
    return out
